# revision 61
# baseline (speedup 1.0000x reference)
"""Trainium2 Bass kernel for nn_Loss_65781719105930 (YOLO-style detection loss).

Strategy (pure data parallelism, 8 cores, 32 images each):
  host:   replicate the reference's target-build scatter (small int64 inputs),
          compact occupied cells, pre-pack aux tables + prediction columns into
          contiguous DMA payloads; gather the target-class logit per
          (cell, anchor) host-side.
  device: dense pass over the 5 conf channels, plus IoU / first-argmax /
          best-anchor-select on compacted tiles; one grouped reduce emits all
          loss partial sums.

Numeric tricks (single scalar-engine act-table set exp_and_others =
{tanh, exp, square}):
  sigmoid(x)      = (1 + tanh(x/2)) / 2   -> work in xi = 2x-1 coords
  sqrt(pred_wh)   = exp((c + ln anchor)/2): the host pre-adds ln(anchor) to
                    the raw w/h channels, so ONE exp activation writes
                    sqrt(pred_wh) straight into the selection matrix
  ln(se)          ~ k*float(bitcast_i32(se)) + b (log2 bit trick); the affine
                    map and the -s_target subtraction fold into the HOST
                    combine: cls = k*sum(fm*lgf) + b*NSLOTS - sum(fm*SAUX)
  noobj dense     = 0.25*sum(1 + tanh(c/2))^2, captured ENTIRELY by the
                    accumulator of one Square activation with bias=1 -- no
                    second dense pass, no accumulator reads beyond it.

Timeline structure (measured):  ~6.3us fixed NEFF init, then sync issues the
three input DMAs on ONE ring in consumption order (the DMA service rate is
aggregate-bandwidth-bound, so the chain-gating fpackA must go first and stay
small); scalar runs exp/tanh chains; vector owns the serial IoU -> argmax ->
select -> reduce chain; gpsimd takes every side chain (union, class adder
tree tail, bits cast, box sq-diffs). Activation biases ship as fpackA
columns -- float biases would emit preamble memsets that delay the DMA
issues. End-of-kernel uses a sem-only barrier (full drains cost ~1us).
"""
import numpy as np

# ---------------------------------------------------------------- constants
NCLS = 20
H = W = 32
HWC = H * W            # 1024 cells/image
A = 5
M = 50
B = 256
CORES = 8
BC = B // CORES        # 32 images per core
CH = A * (5 + NCLS)    # 125 channels
P = 128
LAM_COORD, LAM_OBJ, LAM_NOOBJ, LAM_CLS = 5.0, 1.0, 0.5, 1.0

LN2 = float(np.log(2.0))
LOG_BIAS = 126.94269504   # mean-centering constant for the log2 bit trick
K_LOG = LN2 / (1 << 23)
B_LOG = -LOG_BIAS * LN2
LGF_PAD = 1101004800.0    # float(bitcast_i32(20.0f)): se of an all-zero logit row

_CACHE = {}


def _bf16(x):
    """float32 ndarray -> ml_dtypes.bfloat16 (RNE)."""
    import ml_dtypes
    return np.asarray(x, dtype=np.float32).astype(ml_dtypes.bfloat16)


# ---------------------------------------------------------------- host prep
def _build_target_np(gt_boxes, gt_classes, num_box):
    """Numpy replication of reference.build_target (last object wins, first-max
    class argmax). Returns per-cell [B, HWC] arrays."""
    Bn = gt_boxes.shape[0]
    valid = np.arange(M)[None, :] < num_box[:, None]
    x = gt_boxes[..., 0].astype(np.float32) * H
    y = gt_boxes[..., 1].astype(np.float32) * H
    gx = np.floor(x).astype(np.int64)
    gy = np.floor(y).astype(np.int64)
    flat = np.where(valid, gy * W + gx, HWC)
    bi = np.broadcast_to(np.arange(Bn)[:, None], (Bn, M))

    vals = np.stack([np.ones_like(x), x - gx, y - gy,
                     gt_boxes[..., 2].astype(np.float32) * H,
                     gt_boxes[..., 3].astype(np.float32) * H], axis=-1)
    tgt_box = np.zeros((Bn, HWC + 1, 5), dtype=np.float32)
    tgt_box[bi, flat] = vals
    tgt_cls = np.zeros((Bn, HWC + 1, NCLS), dtype=np.float32)
    tgt_cls[bi, flat, gt_classes.astype(np.int64)] = 1.0

    tgt_box = tgt_box[:, :HWC]
    obj = tgt_box[..., 0]
    cls_t = np.argmax(tgt_cls[:, :HWC], axis=-1).astype(np.int64)
    return obj, tgt_box[..., 1], tgt_box[..., 2], tgt_box[..., 3], tgt_box[..., 4], cls_t


def _split_multi_waits(nc):
    """This container's walrus accepts only ONE sem-wait per instruction; hoist
    extra waits onto standalone NoOps."""
    import concourse.mybir as mybir
    import bass_rust
    n = 0
    for fn in nc.m.functions:
        for blk in fn.blocks:
            new = []
            for ins in blk.instructions:
                si = ins.sync_info
                waits = list(si.on_wait) if si is not None else []
                if len(waits) > 1:
                    for w in waits[:-1]:
                        nop = mybir.InstNoOp(name=f"{ins.name}-w{n}")
                        nop.engine = ins.engine
                        nop.sync_info = bass_rust.SyncInfo(on_wait=[w], on_update=[])
                        new.append(nop)
                        n += 1
                    si.on_wait = [waits[-1]]
                    ins.sync_info = si
                new.append(ins)
            blk.instructions = new
    return n


def _offsets(T):
    """fpackA free-dim offsets. cols_xw chan order is (x, y, conf, w, h)."""
    o = {}
    o["XW"] = 0
    o["B1"] = 25 * T
    o["B2"] = o["B1"] + 2 * T
    o["TAREA"] = o["B2"] + 2 * T
    o["WCONST"] = o["TAREA"] + T
    # consts: [0.0 (act bias), 1.0 (SQ1 bias), bits(-8) (argmax mask), pad]
    o["CZERO"] = o["WCONST"] + 8
    o["CONE"] = o["CZERO"] + 1
    o["CMASK"] = o["CONE"] + 1
    o["SAUX"] = o["CZERO"] + 4
    o["AUX4"] = o["SAUX"] + 5 * T
    o["NFA"] = o["AUX4"] + 4 * T
    return o


# ---------------------------------------------------------------- bass build
def _build_nc(T, split=True):
    """Build the per-core kernel for T cell-blocks per partition (P*T slots).

    DMA payloads (all on sync's ring, in consumption order):
      fpackA [P, NFA] f32: cols_xw (t, ch{x,y,conf,w,h}, a) 25T (w/h have
        +ln(anchor) folded in) | B1, B2 (d{x,y}, t) 2T each | TAREA (t) T |
        WCONST 8 | consts [0, 1, bits(-8), pad] | SAUX (t, a) 5T |
        AUX4 (q{w,h,x,y}, t) 4T
      lgpack [P, 100T] bf16: logits (j, t, a)
      confd  [P, 1280] bf16: dense conf channels

    SRC [P, 8*TA]: q0..5 = (sqrt_w, sqrt_h, x, y, u, uu) per (t, a) |
    q6 = lgf | q7 = SAUX. One selm mult by fmask + one grouped reduce yields
    partials cols 0..7; the dense SQ1 accumulator writes col 8.

    partials out [P, 12] (host applies per-column scales):
      0..3 box (w,h,x,y) sq-diff sums, 4 sum u_sel, 5 sum uu_sel,
      6 sum fmask*lgf, 7 sum fmask*SAUX, 8 sum (1 + ud)^2 dense
    """
    import concourse.bass as bass
    import concourse.mybir as mybir
    import concourse.tile as tile

    f32 = mybir.dt.float32
    bf16 = mybir.dt.bfloat16
    i32 = mybir.dt.int32
    AF = mybir.ActivationFunctionType
    OP = mybir.AluOpType
    AX = mybir.AxisListType

    TA = T * A
    TA2 = TA * 2
    O = _offsets(T)
    NFA = O["NFA"]
    DF = BC * A * HWC // P   # 1280 dense conf elements per partition

    def _v(ap, off, dims):
        """Sub-view of a tile AP: keep its partition dim, replace free dims."""
        return bass.AP(tensor=ap.tensor, offset=ap.offset + off,
                      ap=[list(ap.ap[0])] + dims)

    # Suppress the Bass-init all-engine barrier (it only orders the const-AP
    # memsets, whose single consumer -- the activation bias read -- runs ~4us
    # after them on this kernel's timeline). Saves ~1-2us of prologue.
    import os as _os
    _patch = _os.environ.get("K_KEEP_INIT_BARRIER", "0") != "1"
    _orig_barrier = bass.Bass.all_engine_barrier
    if _patch:
        bass.Bass.all_engine_barrier = lambda self, **kw: None
    try:
        nc = bass.Bass("TRN2")
    finally:
        if _patch:
            bass.Bass.all_engine_barrier = _orig_barrier

    # End-of-kernel: keep the DMA-completion drain + sem clears, but use
    # sequencer-level (sem-only) barriers instead of two full drain+barrier
    # rounds -- the engines' queues are in-order, and the sync drain already
    # waits out every DMA queue.
    from concourse.vector_clock import ScopedClock as _ScopedClock
    _orig_dab = tile.TileContext._drain_and_barrier
    if _os.environ.get("K_KEEP_END_BARRIER", "0") != "1":
        def _light_dab(self, tick_clock, wait_clock):
            drain_inst = self.nc.sync.drain()
            wait_clock.add_sem_waits(
                drain_inst.ins,
                _ScopedClock({None: tick_clock.global_clock}))
            self.nc.all_engine_barrier(sem_only=True)
            popped = self.nc._tile_sem_poison_stack.pop()
            assert popped is self._sem_poison
            self.nc.clear_and_free_semaphores(
                list(self.sems.allocated().values()))
        tile.TileContext._drain_and_barrier = _light_dab
    fpackA_d = nc.declare_dram_parameter("fpackA", [P, NFA], f32, isOutput=False)
    lgpack_d = nc.declare_dram_parameter("lgpack", [P, 100 * T], bf16, isOutput=False)
    confd_d = nc.declare_dram_parameter("confd", [P, DF], bf16, isOutput=False)
    partials_d = nc.declare_dram_parameter("partials", [P, 12], f32, isOutput=True)

    with tile.TileContext(nc) as tc:
        with tc.tile_pool(name="sb", bufs=1) as pool:
            # SRC: (q, t, a), q in {w, h, x, y, u, uu, lgf, SAUX}
            SRC = pool.tile([P, 8 * TA], f32, name="SRC")

            # ---------------- input DMAs
            # all three on sync's single ring, in consumption order (the
            # service rate is aggregate-bandwidth-bound, so the first DMA
            # must stay small -- it gates the whole compute chain); a second
            # ring makes the engines round-robin between queues, stalling
            # fpackA mid-transfer
            fp = pool.tile([P, NFA], f32, name="fp")
            nc.sync.dma_start(out=fp[:], in_=fpackA_d[:])
            lg_in = pool.tile([P, 100 * T], bf16, name="lg_in")
            nc.sync.dma_start(out=lg_in[:], in_=lgpack_d[:])
            confd = pool.tile([P, DF], bf16, name="confd")
            nc.sync.dma_start(out=confd[:], in_=confd_d[:])

            partials = pool.tile([P, 12], f32, name="partials")

            # ---------------- scalar engine queue (one act-table set)
            # biases come from fpackA as APs -- a float bias would emit a
            # const-AP memset into the preamble, delaying the DMA issues
            bias0 = _v(fp[:], O["CZERO"], [[1, 1]])
            bias1 = _v(fp[:], O["CONE"], [[1, 1]])
            # sh = sqrt(pred_wh) = exp((chan{w,h} + ln anchor)/2) -> SRC q0,q1
            # directly: the host pre-adds ln(anchor) to the w/h channels
            nc.scalar.activation(
                _v(SRC[:], 0, [[TA, 2], [A, T], [1, A]]),
                _v(fp[:], O["XW"] + 15, [[5, 2], [25, T], [1, A]]),
                AF.Exp, bias=bias0, scale=0.5)
            # x, y, u = tanh(chan{x,y,conf}/2) -> SRC q2, q3, q4
            nc.scalar.activation(
                _v(SRC[:], 2 * TA, [[TA, 3], [A, T], [1, A]]),
                _v(fp[:], O["XW"], [[5, 3], [25, T], [1, A]]),
                AF.Tanh, bias=bias0, scale=0.5)
            # e = exp(logits), bf16; host packs logits as (j, t, a) so the
            # class adder tree below runs on cheap flat views
            e = pool.tile([P, 100 * T], bf16, name="e")
            nc.scalar.activation(
                _v(e[:], 0, [[1, 100 * T]]),
                _v(lg_in[:], 0, [[1, 100 * T]]),
                AF.Exp, bias=bias0)
            # dense pass, pinned after e in the scalar queue (sim-only hint):
            # ud = tanh(c/2) bf16, then SQ1 = (1 + ud)^2 whose accumulator is
            # the ONLY dense quantity the host needs:
            #   sum(1+ud)^2 = N + 2 sum ud + sum ud^2 = 4 * sum sigmoid^2
            UD = pool.tile([P, DF], bf16, name="UD")
            SQ1 = pool.tile([P, DF], bf16, name="SQ1")
            with tc.tile_wait_until(0.05):
                nc.scalar.activation(UD[:], confd[:], AF.Tanh, bias=bias0,
                                     scale=0.5)
                nc.scalar.activation(SQ1[:], UD[:], AF.Square, bias=bias1,
                                     accum_out=_v(partials[:], 8, [[1, 1]]))

            # ---------------- vector/gpsimd, emitted in dataflow order
            tcnt = [0]

            def tmp(n, dtype=f32):
                tcnt[0] += 1
                return pool.tile([P, n], dtype, name=f"t{tcnt[0]}")

            # wfull = sh*sh = pred_wh (xi-space half-width), (d, t, a)
            wf = tmp(TA2)
            SH2 = _v(SRC[:], 0, [[TA, 2], [1, TA]])
            WF = _v(wf[:], 0, [[TA, 2], [1, TA]])
            nc.vector.tensor_tensor(out=WF, in0=SH2, in1=SH2, op=OP.mult)

            # gpsimd: SAUX -> SRC q7; uu = u^2 -> SRC q5
            nc.gpsimd.tensor_copy(out=_v(SRC[:], 7 * TA, [[1, TA]]),
                                  in_=_v(fp[:], O["SAUX"], [[1, TA]]))
            nc.gpsimd.tensor_tensor(out=_v(SRC[:], 5 * TA, [[1, TA]]),
                                    in0=_v(SRC[:], 4 * TA, [[1, TA]]),
                                    in1=_v(SRC[:], 4 * TA, [[1, TA]]),
                                    op=OP.mult)
            # gpsimd union branch (no STT on Pool in this walrus):
            # 4*areaA as (2w)(2h); u1 = areaA4 + TAREA
            wf2 = tmp(TA2)
            nc.gpsimd.tensor_tensor(out=wf2[:], in0=wf[:], in1=wf[:], op=OP.add)
            areaA4 = tmp(TA)
            nc.gpsimd.tensor_tensor(out=areaA4[:], in0=_v(wf2[:], 0, [[1, TA]]),
                                    in1=_v(wf2[:], TA, [[1, TA]]), op=OP.mult)
            u1 = tmp(TA)
            nc.gpsimd.tensor_tensor(out=_v(u1[:], 0, [[A, T], [1, A]]),
                                    in0=_v(areaA4[:], 0, [[A, T], [1, A]]),
                                    in1=_v(fp[:], O["TAREA"], [[1, T], [0, A]]),
                                    op=OP.add)

            # IoU in xi coords. XY = SRC q2, q3 as (d, t, a)
            XY = _v(SRC[:], 2 * TA, [[TA, 2], [1, TA]])
            lo = tmp(TA2)
            nc.vector.tensor_tensor(out=lo[:], in0=XY, in1=WF, op=OP.subtract)
            hi = tmp(TA2)
            nc.vector.tensor_tensor(out=hi[:], in0=XY, in1=WF, op=OP.add)

            # gpsimd box sq-diffs per (t, a) BEFORE selection (bit-identical
            # under the one-hot fmask); emitted after lo/hi so the in-place
            # overwrite of SRC q0..3 orders after their XY reads
            D4 = tmp(4 * TA)
            nc.gpsimd.tensor_tensor(out=_v(D4[:], 0, [[TA, 4], [A, T], [1, A]]),
                                    in0=_v(SRC[:], 0, [[TA, 4], [A, T], [1, A]]),
                                    in1=_v(fp[:], O["AUX4"],
                                           [[T, 4], [1, T], [0, A]]),
                                    op=OP.subtract)
            nc.gpsimd.tensor_tensor(out=_v(SRC[:], 0, [[1, 4 * TA]]),
                                    in0=_v(D4[:], 0, [[1, 4 * TA]]),
                                    in1=_v(D4[:], 0, [[1, 4 * TA]]),
                                    op=OP.mult)
            B1v = _v(fp[:], O["B1"], [[T, 2], [1, T], [0, A]])
            B2v = _v(fp[:], O["B2"], [[T, 2], [1, T], [0, A]])
            LOv = _v(lo[:], 0, [[TA, 2], [A, T], [1, A]])
            HIv = _v(hi[:], 0, [[TA, 2], [A, T], [1, A]])
            t1 = tmp(TA2)
            nc.vector.tensor_tensor(out=_v(t1[:], 0, [[TA, 2], [A, T], [1, A]]),
                                    in0=HIv, in1=B2v, op=OP.min)
            t2 = tmp(TA2)
            nc.vector.tensor_tensor(out=_v(t2[:], 0, [[TA, 2], [A, T], [1, A]]),
                                    in0=LOv, in1=B1v, op=OP.max)
            t3 = tmp(TA2)
            nc.vector.tensor_tensor(out=t3[:], in0=t1[:], in1=t2[:], op=OP.subtract)
            # iw/ih = max(t3, 0); inter = iw * ih
            iwih = tmp(TA2)
            nc.vector.tensor_scalar_max(iwih[:], t3[:], 0.0)
            inter = tmp(TA)
            nc.vector.tensor_tensor(out=inter[:], in0=_v(iwih[:], 0, [[1, TA]]),
                                    in1=_v(iwih[:], TA, [[1, TA]]), op=OP.mult)
            # u2 = u1 - inter, fused
            u2 = tmp(TA)
            nc.vector.scalar_tensor_tensor(
                out=u2[:], in0=inter[:], scalar=-1.0, in1=u1[:],
                op0=OP.mult, op1=OP.add)
            # no native divide on DVE, and the custom-DVE approx reciprocals
            # fail this walrus' codegen: plain reciprocal + mult
            iou = tmp(TA)
            rcp = tmp(TA)
            nc.vector.reciprocal(out=rcp[:], in_=u2[:])
            nc.vector.tensor_tensor(out=iou[:], in0=inter[:], in1=rcp[:],
                                    op=OP.mult)

            # class path: se = sum_j e[j, (t,a)] via a split adder tree over
            # the j-major layout -- every level is a FLAT add. First (big)
            # level on vector bf16; the rest + the bits cast hide on gpsimd.
            s1 = tmp(10 * TA, bf16)
            nc.vector.tensor_tensor(out=_v(s1[:], 0, [[1, 10 * TA]]),
                                    in0=_v(e[:], 0, [[1, 10 * TA]]),
                                    in1=_v(e[:], 10 * TA, [[1, 10 * TA]]),
                                    op=OP.add)
            s2 = tmp(5 * TA, bf16)
            nc.gpsimd.tensor_tensor(out=_v(s2[:], 0, [[1, 5 * TA]]),
                                    in0=_v(s1[:], 0, [[1, 5 * TA]]),
                                    in1=_v(s1[:], 5 * TA, [[1, 5 * TA]]),
                                    op=OP.add)

            # first-argmax over a via bit-packed keys: clear the low 3
            # mantissa bits of iou (all ious >= 0, so int order == float
            # order) and OR in a per-anchor tiebreak (7 - a), fused in one
            # STT (the -8 mask ships in fpackA -- imm ints lower as f32).
            # Ties -- exact at 8-ULP granularity -- resolve to the FIRST
            # anchor, matching jnp.argmax. Padding cells select anchor 0.
            ipk2 = tmp(TA, i32)
            nc.vector.scalar_tensor_tensor(
                out=_v(ipk2[:], 0, [[A, T], [1, A]]),
                in0=_v(iou[:], 0, [[A, T], [1, A]]).bitcast(i32),
                scalar=_v(fp[:], O["CMASK"], [[1, 1]]).bitcast(i32),
                in1=_v(fp[:], O["WCONST"], [[0, T], [1, A]]).bitcast(i32),
                op0=OP.bitwise_and, op1=OP.bitwise_or)
            rmax = tmp(T, i32)
            nc.vector.tensor_reduce(out=rmax[:],
                                    in_=_v(ipk2[:], 0, [[A, T], [1, A]]),
                                    axis=AX.X, op=OP.max)
            fmask = tmp(TA)
            nc.vector.tensor_tensor(out=_v(fmask[:], 0, [[A, T], [1, A]]),
                                    in0=_v(ipk2[:], 0, [[A, T], [1, A]]),
                                    in1=_v(rmax[:], 0, [[1, T], [0, A]]),
                                    op=OP.is_equal)

            # gpsimd class-path tail: remaining adder levels, then
            # lgf = float(bits(se)) -> SRC q6 (host applies the affine log
            # map and the SAUX subtraction)
            s3 = tmp(2 * TA)
            nc.gpsimd.tensor_tensor(out=_v(s3[:], 0, [[1, 2 * TA]]),
                                    in0=_v(s2[:], 0, [[1, 2 * TA]]),
                                    in1=_v(s2[:], 2 * TA, [[1, 2 * TA]]),
                                    op=OP.add)
            s4 = tmp(TA)
            nc.gpsimd.tensor_tensor(out=_v(s4[:], 0, [[1, TA]]),
                                    in0=_v(s3[:], 0, [[1, TA]]),
                                    in1=_v(s3[:], TA, [[1, TA]]),
                                    op=OP.add)
            se = tmp(TA)
            nc.gpsimd.tensor_tensor(out=_v(se[:], 0, [[1, TA]]),
                                    in0=_v(s4[:], 0, [[1, TA]]),
                                    in1=_v(s2[:], 4 * TA, [[1, TA]]),
                                    op=OP.add)
            nc.gpsimd.tensor_copy(out=_v(SRC[:], 6 * TA, [[1, TA]]),
                                  in_=se[:].bitcast(i32))

            selm = pool.tile([P, 8 * TA], f32, name="selm")
            nc.vector.tensor_tensor(out=_v(selm[:], 0, [[TA, 8], [1, TA]]),
                                    in0=_v(SRC[:], 0, [[TA, 8], [1, TA]]),
                                    in1=_v(fmask[:], 0, [[0, 8], [1, TA]]),
                                    op=OP.mult)
            nc.vector.tensor_reduce(out=_v(partials[:], 0, [[1, 8]]),
                                    in_=_v(selm[:], 0, [[TA, 8], [1, TA]]),
                                    axis=AX.X, op=OP.add)

            nc.sync.dma_start(out=partials_d[:], in_=partials[:],
                              single_packet=True)

    tile.TileContext._drain_and_barrier = _orig_dab
    if split:
        _split_multi_waits(nc)
    return nc


# -------------------------------------------------------------- shard builder
def _make_in_maps(out, gt_boxes, anchor_np, gt_classes_np, num_box_np, T):
    obj, xo, yo, tw, th, cls_t = _build_target_np(gt_boxes, gt_classes_np,
                                                  num_box_np)
    SLOTS = P * T
    TA = T * A
    out_r = out.reshape(B, A, 25, HWC)
    sqa = np.sqrt(anchor_np)                       # [A, 2]

    in_maps = []
    for c in range(CORES):
        sl = slice(c * BC, (c + 1) * BC)
        ob = obj[sl]                               # [BC, HWC]
        bloc, hwloc = np.nonzero(ob > 0)
        K = len(bloc)
        assert K <= SLOTS

        def place(vals):
            buf = np.zeros(SLOTS, dtype=np.float32)
            buf[:K] = vals
            return buf.reshape(P, T)

        xov = place(xo[sl][bloc, hwloc])
        yov = place(yo[sl][bloc, hwloc])
        twv = place(tw[sl][bloc, hwloc])
        thv = place(th[sl][bloc, hwloc])

        # occupied-cell prediction channels [K, A, 25]
        colsb = np.zeros((SLOTS, A, 25), dtype=np.float32)
        if K:
            colsb[:K] = out_r[sl].transpose(0, 3, 1, 2)[bloc, hwloc]
        # cols_xw (t, ch{x,y,conf,w,h}, a); w/h get +ln(anchor) so the device
        # exp((c + ln A)/2) yields sqrt(pred_wh) directly
        sel = np.ascontiguousarray(colsb[:, :, [21, 22, 20, 23, 24]])
        sel[:, :, 3] += np.log(anchor_np[None, :, 0])
        sel[:, :, 4] += np.log(anchor_np[None, :, 1])
        cols_xw = np.ascontiguousarray(
            sel.reshape(P, T, A, 5).transpose(0, 1, 3, 2)).reshape(P, 25 * T)
        # logits packed (j, t, a): flat adder-tree levels on device
        logits = np.ascontiguousarray(
            colsb[:, :, :20].reshape(P, T * A, NCLS).transpose(0, 2, 1)
        ).reshape(P, 100 * T)

        # target-class logit per (t, a); padding slots get k*lgf_pad + b so
        # the host-side cls combine cancels them exactly
        clsv = place(cls_t[sl][bloc, hwloc].astype(np.float32)).astype(np.int64)
        s_aux = np.take_along_axis(
            colsb[:, :, :20].reshape(SLOTS, A, 20),
            clsv.reshape(SLOTS, 1, 1).repeat(A, axis=1), axis=2
        )[:, :, 0].astype(np.float32)
        s_aux[K:] = np.float32(K_LOG * LGF_PAD + B_LOG)
        s_aux = s_aux.reshape(P, TA)

        # xi-space target box edges (d{x,y}, t): center 2o-1, half-width t_wh
        cxv = 2.0 * xov - 1.0
        cyv = 2.0 * yov - 1.0
        b1 = np.stack([cxv - twv, cyv - thv], axis=1).reshape(P, 2 * T)
        b2 = np.stack([cxv + twv, cyv + thv], axis=1).reshape(P, 2 * T)
        tarea = (4.0 * twv * thv).reshape(P, T)

        # argmax tiebreak bits (7 - a) as raw int32, shipped bitcast as f32
        wconst = np.zeros(8, dtype=np.int32)
        wconst[:A] = 7 - np.arange(A, dtype=np.int32)
        wconst = np.broadcast_to(wconst.view(np.float32), (P, 8))

        # AUX4 in q-order (w, h, x, y); padding rows get the exact padded
        # prediction (sqrt(anchor0), tanh(0)=0) so their sq-diffs vanish
        aux4 = np.stack([np.sqrt(twv), np.sqrt(thv), cxv, cyv], axis=1)
        a4f = aux4.transpose(0, 2, 1).reshape(SLOTS, 4)
        a4f[K:] = [float(sqa[0, 0]), float(sqa[0, 1]), 0.0, 0.0]
        aux4 = np.ascontiguousarray(
            a4f.reshape(P, T, 4).transpose(0, 2, 1)).reshape(P, 4 * T)

        consts = np.broadcast_to(np.array(
            [0.0, 1.0, float(np.int32(-8).view(np.float32)), 0.0],
            dtype=np.float32), (P, 4))
        fpackA = np.concatenate(
            [cols_xw, b1, b2, tarea, wconst, consts, s_aux, aux4],
            axis=1)

        # dense conf channels: [BC, A, HWC] -> [P, 1280] bf16
        confd = out_r[sl][:, :, 20, :].reshape(P, -1)

        in_maps.append({
            "fpackA": np.ascontiguousarray(fpackA, dtype=np.float32),
            "lgpack": _bf16(logits),
            "confd": _bf16(confd),
        })
    return in_maps


# ---------------------------------------------------------------- entry point
def kernel(out, gt_boxes, anchor, gt_classes, num_box):
    from concourse.bass_utils import run_bass_kernel_spmd

    out = np.ascontiguousarray(np.asarray(out, dtype=np.float32))
    gt_boxes = np.asarray(gt_boxes, dtype=np.float32)
    anchor_np = np.asarray(anchor, dtype=np.float32)
    gt_classes_np = np.asarray(gt_classes)
    num_box_np = np.asarray(num_box)

    # per-core occupied-cell counts decide the compiled tile factor T
    obj = _build_target_np(gt_boxes, gt_classes_np, num_box_np)[0]
    ks = [int((obj[c * BC:(c + 1) * BC] > 0).sum()) for c in range(CORES)]
    maxk = max(ks)
    T = max(1, -(-maxk // P))
    assert maxk <= 13 * P and T <= 13

    in_maps = _make_in_maps(out, gt_boxes, anchor_np, gt_classes_np,
                            num_box_np, T)

    import os
    key = f"nc{T}"
    if key not in _CACHE:
        _CACHE[key] = _build_nc(T)
    trace = os.environ.get("KERNEL_TRACE", "0") == "1"
    res = None
    for attempt in range(3):
        try:
            res = run_bass_kernel_spmd(_CACHE[key], in_maps,
                                       core_ids=list(range(CORES)), trace=trace)
            break
        except Exception:
            # transient device/runtime hiccups (e.g. NRT_EXEC_UNIT_UNRECOVERABLE)
            # recover on retry; re-raise only if persistent
            if attempt == 2:
                raise
            import time
            time.sleep(2.0)
    if trace:
        print(f"HW exec time: {res.exec_time_ns} ns  (mean {res.mean_exec_time_ns})")

    cols = np.zeros(12, dtype=np.float64)
    for c in range(CORES):
        cols += res.results[c]["partials"].astype(np.float64).sum(axis=0)
    K = float(sum(ks))
    NSLOTS = float(CORES * P * T)
    NDENSE = float(B * A * HWC)
    box_loss = np.float32(LAM_COORD / B * (cols[0] + cols[1]
                                            + 0.25 * (cols[2] + cols[3])))
    conf_loss = np.float32(LAM_OBJ / B * 0.25 * (cols[5] - 2.0 * cols[4] + K))
    # col 8 = sum (1+ud)^2 over the dense set = 4 * sum sigmoid^2
    noobj_loss = np.float32(LAM_NOOBJ / B * 0.25 * (
        cols[8] - (cols[5] + 2.0 * cols[4] + K)))
    cls_loss = np.float32(LAM_CLS / B * (K_LOG * cols[6] + B_LOG * NSLOTS
                                         - cols[7]))
    return (box_loss, conf_loss, noobj_loss, cls_loss)
